# revision 12
# baseline (speedup 1.0000x reference)
"""Trainium2 Bass kernel for Tucker-factored multi-head attention.

Problem: x[B=4, P=32, Q=32, E1=8, E2=8, E3=12], mode-wise (Kronecker)
Q/K/V/O projections, 12 heads (2x2x3) of dim 64, softmax attention over
the 1024 tokens of each batch element.

Sharding over 8 cores: core c handles batch b=c//2 and the 6 heads with
h1=c%2 (the head axes factor as (h1,h2,h3)=(2,2,3); per-head projection
matrices are Kronecker column/row slices, precomputed densely on host).
Each core emits a partial output (its heads' contribution through the
output projection); the host sums the two cores sharing each batch.

Numerics: projections fp32 on the PE; S computed exactly via the
bf16-split 2-matmul trick ([qh;ql]x[kh;kl] + [qh;ql]x[kl;kh] == full
fp32 product to ~5e-6); softmax row-max via DVE, exp on ACT with fused
per-partition bias and denominator accumulation; attention weights and
V/O projections in bf16 (error budget ~3e-3 << 2e-2).
"""
import numpy as np
import ml_dtypes
from contextlib import ExitStack

import concourse.bass as bass
import concourse.tile as tile
from concourse import bacc, mybir
from concourse.bass_utils import run_bass_kernel_spmd

F32 = mybir.dt.float32
BF16 = mybir.dt.bfloat16

B, P, Qd = 4, 32, 32
E1, E2, E3 = 8, 8, 12
H1, H2, H3 = 2, 2, 3
T = P * Qd            # 1024 tokens
E = E1 * E2 * E3      # 768 features
HD = 64               # per-head dim
NH = 6                # heads per core
SCALE = float(HD) ** -0.5
N_CORES = 8

_module_cache = {}


def build_module(dbg=False):
    nc = bacc.Bacc("TRN2", target_bir_lowering=False, debug=False)

    x_d = nc.dram_tensor("x", [T, E], F32, kind="ExternalInput").ap()
    wq_d = nc.dram_tensor("wq", [E, NH * HD], F32, kind="ExternalInput").ap()
    wk_d = nc.dram_tensor("wk", [E, NH * HD], F32, kind="ExternalInput").ap()
    wv_d = nc.dram_tensor("wv", [E, NH * HD], BF16, kind="ExternalInput").ap()
    wo_d = nc.dram_tensor("wo", [NH * HD, E], BF16, kind="ExternalInput").ap()
    id_d = nc.dram_tensor("ident", [128, 128], F32, kind="ExternalInput").ap()
    out_d = nc.dram_tensor("out", [T, E], F32, kind="ExternalOutput").ap()
    KC = E // 128         # 6 feature chunks
    TC = T // 128         # 8 token chunks
    MC = NH * HD // 128   # 3 output chunks for q/k/v
    dbg_d = {}
    if dbg:
        for nm, shp, dt in [("d_xt", [128, KC * T], F32),
                            ("d_qst0", [128, T], BF16), ("d_ksta0", [128, T], BF16),
                            ("d_kstb0", [128, T], BF16), ("d_v0", [128, NH * HD], BF16),
                            ("d_se0", [128, TC * T], BF16), ("d_set0", [128, TC * T], BF16),
                            ("d_nmx0", [128, TC], F32), ("d_den0", [128, TC], F32),
                            ("d_bcs0", [64, T], BF16), ("d_ot0", [128, T], BF16),
                            ("d_ot1", [128, T], BF16), ("d_ot2", [128, T], BF16)]:
            dbg_d[nm] = nc.dram_tensor(nm, shp, dt, kind="ExternalOutput").ap()


    with tile.TileContext(nc) as tc, ExitStack() as ctx:
        # ---------- persistent SBUF ----------
        wpool = ctx.enter_context(tc.tile_pool(name="weights", bufs=1))
        main = ctx.enter_context(tc.tile_pool(name="main", bufs=1))

        idt_f = wpool.tile([128, 128], F32)
        nc.sync.dma_start(idt_f[:], id_d[:])
        idt_b = wpool.tile([128, 128], BF16)
        nc.vector.tensor_copy(idt_b[:], idt_f[:])
        ones_b = wpool.tile([1, HD], BF16)
        nc.vector.memset(ones_b[:], 1.0)

        wq_sb = [wpool.tile([128, NH * HD], F32, name=f"wq{k}") for k in range(KC)]
        wk_sb = [wpool.tile([128, NH * HD], F32, name=f"wk{k}") for k in range(KC)]
        wv_sb = [wpool.tile([128, NH * HD], BF16, name=f"wv{k}") for k in range(KC)]
        wo_sb = [wpool.tile([128, E], BF16, name=f"wo{m}") for m in range(MC)]
        for k in range(KC):
            nc.sync.dma_start(wq_sb[k][:], wq_d[k * 128:(k + 1) * 128, :])
            nc.sync.dma_start(wk_sb[k][:], wk_d[k * 128:(k + 1) * 128, :])
            nc.sync.dma_start(wv_sb[k][:], wv_d[k * 128:(k + 1) * 128, :])
        for m in range(MC):
            nc.sync.dma_start(wo_sb[m][:], wo_d[m * 128:(m + 1) * 128, :])

        # xt: transposed x, one tensor, feature-chunk k at cols [k*1024, +1024)
        xt = main.tile([128, KC * T], F32)
        xt3 = xt[:].rearrange("p (k t) -> p k t", k=KC)
        xth = main.tile([128, KC * T], BF16)

        # per-head stacked bf16 split tiles
        qst = [main.tile([128, T], BF16, name=f"qst{h}") for h in range(NH)]
        ksta = [main.tile([128, T], BF16, name=f"ksta{h}") for h in range(NH)]
        kstb = [main.tile([128, T], BF16, name=f"kstb{h}") for h in range(NH)]
        v_sb = [main.tile([128, NH * HD], BF16, name=f"vsb{t}") for t in range(TC)]
        otall = [main.tile([128, T], BF16, name=f"ot{m}") for m in range(MC)]

        # ---------- PSUM pools ----------
        ps_big = ctx.enter_context(
            tc.tile_pool(name="ps_big", bufs=3, space="PSUM"))
        ps_small = ctx.enter_context(
            tc.tile_pool(name="ps_small", bufs=2, space="PSUM"))

        # ---------- stage 0/1: load x, transpose to xt ----------
        with tc.tile_pool(name="xin", bufs=3) as xin:
            for tc_i in range(TC):
                xtile = xin.tile([128, E], F32, tag="x")
                nc.sync.dma_start(xtile[:], x_d[tc_i * 128:(tc_i + 1) * 128, :])
                trp = ps_big.tile([128, E], F32, tag="big")
                for k in range(KC):
                    nc.tensor.transpose(
                        trp[:, k * 128:(k + 1) * 128],
                        xtile[:, k * 128:(k + 1) * 128], idt_f[:])
                trp3 = trp[:].rearrange("p (k t) -> p k t", k=KC)
                nc.vector.tensor_copy(
                    xt3[:, :, tc_i * 128:(tc_i + 1) * 128], trp3[:])
        nc.scalar.copy(xth[:], xt[:])
        if dbg:
            nc.sync.dma_start(dbg_d["d_xt"][:], xt[:])

        # ---------- stage 2: Q/K projections (fp32) + bf16 split ----------
        for which, w_sb, dst_a, dst_b in (
                ("q", wq_sb, qst, None), ("k", wk_sb, ksta, kstb)):
            for m in range(MC):
                pp = ps_big.tile([128, T], F32, tag="big")
                for nh in range(2):
                    ncol = slice(nh * 512, (nh + 1) * 512)
                    for k in range(KC):
                        nc.tensor.matmul(
                            pp[:, ncol],
                            w_sb[k][:, m * 128:(m + 1) * 128],
                            xt3[:, k, ncol],
                            start=(k == 0), stop=(k == KC - 1))
                for hh in range(2):
                    h = 2 * m + hh
                    rows = slice(hh * 64, hh * 64 + 64)
                    # high half: bf16 round (ACT)
                    nc.scalar.copy(dst_a[h][0:64, :], pp[rows, :])
                    # low half: residual (DVE)
                    nc.vector.tensor_tensor(
                        dst_a[h][64:128, :], pp[rows, :], dst_a[h][0:64, :],
                        op=mybir.AluOpType.subtract)
                    if dst_b is not None:
                        nc.vector.tensor_copy(dst_b[h][0:64, :],
                                              dst_a[h][64:128, :])
                        nc.vector.tensor_copy(dst_b[h][64:128, :],
                                              dst_a[h][0:64, :])

        if dbg:
            nc.sync.dma_start(dbg_d["d_qst0"][:], qst[0][:])
            nc.sync.dma_start(dbg_d["d_ksta0"][:], ksta[0][:])
            nc.sync.dma_start(dbg_d["d_kstb0"][:], kstb[0][:])

        # ---------- stage 3: V projection (bf16) ----------
        for t_i in range(TC):
            pv = ps_small.tile([128, NH * HD], F32, tag="small")
            for k in range(KC):
                nc.tensor.matmul(
                    pv[:],
                    xth[:, k * T + t_i * 128: k * T + t_i * 128 + 128],
                    wv_sb[k][:],
                    start=(k == 0), stop=(k == KC - 1))
            nc.vector.tensor_copy(v_sb[t_i][:], pv[:])
            if dbg and t_i == 0:
                nc.sync.dma_start(dbg_d["d_v0"][:], v_sb[0][:])

        # ---------- stage 4: per-head attention ----------
        with tc.tile_pool(name="hloop", bufs=2) as hpool, \
                tc.tile_pool(name="sepool", bufs=2) as sepool:
            for h in range(NH):
                nmx = hpool.tile([128, TC], F32, tag="nmx")
                den = hpool.tile([128, TC], F32, tag="den")
                se = sepool.tile([128, TC * T], BF16, tag="se")
                se3 = se[:].rearrange("p (q t) -> p q t", q=TC)
                set_t = sepool.tile([128, TC * T], BF16, tag="set")
                set3 = set_t[:].rearrange("p (c t) -> p c t", c=TC)

                for qc in range(TC):
                    sp = ps_big.tile([128, T], F32, tag="big")
                    for nh in range(2):
                        ncol = slice(nh * 512, (nh + 1) * 512)
                        nc.tensor.matmul(sp[:, ncol],
                                         qst[h][:, qc * 128:(qc + 1) * 128],
                                         ksta[h][:, ncol],
                                         start=True, stop=False)
                        nc.tensor.matmul(sp[:, ncol],
                                         qst[h][:, qc * 128:(qc + 1) * 128],
                                         kstb[h][:, ncol],
                                         start=False, stop=True)
                    nc.vector.tensor_reduce(
                        nmx[:, qc:qc + 1], sp[:], axis=mybir.AxisListType.X,
                        op=mybir.AluOpType.max, negate=True)
                    nc.scalar.activation(
                        se3[:, qc, :], sp[:], mybir.ActivationFunctionType.Exp,
                        bias=nmx[:, qc:qc + 1], scale=1.0,
                        accum_out=den[:, qc:qc + 1])
                    # transpose exp'd tile into set
                    stp = ps_small.tile([128, T], BF16, tag="small")
                    for c in range(TC):
                        nc.tensor.transpose(
                            stp[:, c * 128:(c + 1) * 128],
                            se3[:, qc, c * 128:(c + 1) * 128], idt_b[:])
                    stp3 = stp[:].rearrange("p (c t) -> p c t", c=TC)
                    nc.vector.tensor_copy(
                        set3[:, :, qc * 128:(qc + 1) * 128], stp3[:])

                if dbg and h == 0:
                    nc.sync.dma_start(dbg_d["d_se0"][:], se[:])
                    nc.sync.dma_start(dbg_d["d_set0"][:], set_t[:])
                    nc.sync.dma_start(dbg_d["d_nmx0"][:], nmx[:])
                    nc.sync.dma_start(dbg_d["d_den0"][:], den[:])

                # denominators -> bcast of reciprocal, [64, T] bf16
                rden = hpool.tile([128, TC], F32, tag="rden")
                nc.vector.reciprocal(rden[:], den[:])
                rdp = ps_big.tile([1, T], F32, tag="big")
                for qc in range(TC):
                    nc.tensor.transpose(rdp[:, qc * 128:(qc + 1) * 128],
                                        rden[:, qc:qc + 1], idt_f[:])
                rdt = hpool.tile([1, T], BF16, tag="rdt")
                nc.scalar.copy(rdt[:], rdp[:])
                bcp = ps_big.tile([64, T], F32, tag="big")
                for nh in range(2):
                    ncol = slice(nh * 512, (nh + 1) * 512)
                    nc.tensor.matmul(bcp[:, ncol], ones_b[:], rdt[:, ncol],
                                     start=True, stop=True)
                bcs = hpool.tile([64, T], BF16, tag="bcs")
                nc.scalar.copy(bcs[:], bcp[:])

                # AV: V-stationary, out ot [64, T] fp32 psum
                otp = ps_big.tile([64, T], F32, tag="big")
                for nh in range(2):
                    ncol = slice(nh * 512, (nh + 1) * 512)
                    for c in range(TC):
                        nc.tensor.matmul(
                            otp[:, ncol], v_sb[c][:, h * 64:(h + 1) * 64],
                            set3[:, c, ncol],
                            start=(c == 0), stop=(c == TC - 1))
                if dbg and h == 0:
                    nc.sync.dma_start(dbg_d["d_bcs0"][:], bcs[:])
                # normalize + store to otall
                nc.vector.tensor_tensor(
                    otall[h // 2][(h % 2) * 64:(h % 2) * 64 + 64, :],
                    otp[:], bcs[:], op=mybir.AluOpType.mult)

        if dbg:
            nc.sync.dma_start(dbg_d["d_ot0"][:], otall[0][:])
            nc.sync.dma_start(dbg_d["d_ot1"][:], otall[1][:])
            nc.sync.dma_start(dbg_d["d_ot2"][:], otall[2][:])

        # ---------- stage 5: output projection ----------
        with tc.tile_pool(name="outp", bufs=3) as opool:
            for t_i in range(TC):
                fp = ps_big.tile([128, E], F32, tag="big")
                for ncol in (slice(0, 512), slice(512, 768)):
                    for m in range(MC):
                        nc.tensor.matmul(
                            fp[:, ncol], otall[m][:, t_i * 128:(t_i + 1) * 128],
                            wo_sb[m][:, ncol], start=(m == 0), stop=(m == MC - 1))
                ob = opool.tile([128, E], F32, tag="ob")
                nc.scalar.copy(ob[:], fp[:])
                nc.sync.dma_start(out_d[t_i * 128:(t_i + 1) * 128, :], ob[:])

    nc.compile()
    return nc


def _head_w(w1, w2, w3, h1, h2, h3):
    return np.kron(w1[:, h1::H1], np.kron(w2[:, h2::H2], w3[:, h3::H3]))


def _head_wo(w1, w2, w3, h1, h2, h3):
    return np.kron(w1[h1::H1, :], np.kron(w2[h2::H2, :], w3[h3::H3, :]))


def _prep_weights(h1, w1q, w2q, w3q, w1k, w2k, w3k, w1v, w2v, w3v,
                  w1o, w2o, w3o):
    heads = [(h2, h3) for h2 in range(H2) for h3 in range(H3)]
    wq = np.concatenate(
        [_head_w(w1q, w2q, w3q, h1, h2, h3) for h2, h3 in heads], axis=1)
    wk = np.concatenate(
        [_head_w(w1k, w2k, w3k, h1, h2, h3) for h2, h3 in heads], axis=1)
    wv = np.concatenate(
        [_head_w(w1v, w2v, w3v, h1, h2, h3) for h2, h3 in heads], axis=1)
    wo = np.concatenate(
        [_head_wo(w1o, w2o, w3o, h1, h2, h3) for h2, h3 in heads], axis=0)
    return ((SCALE * wq).astype(np.float32), wk.astype(np.float32),
            wv.astype(ml_dtypes.bfloat16), wo.astype(ml_dtypes.bfloat16))


def kernel(x, w1q, w2q, w3q, w1k, w2k, w3k, w1v, w2v, w3v, w1o, w2o, w3o):
    if "nc" not in _module_cache:
        _module_cache["nc"] = build_module()
    nc = _module_cache["nc"]

    ws = [np.asarray(w, np.float32) for w in
          (w1q, w2q, w3q, w1k, w2k, w3k, w1v, w2v, w3v, w1o, w2o, w3o)]
    wsets = [_prep_weights(h1, *ws) for h1 in range(H1)]
    xf = np.ascontiguousarray(np.asarray(x, np.float32).reshape(B, T, E))
    ident = np.eye(128, dtype=np.float32)

    in_maps = []
    for core in range(N_CORES):
        b, h1 = core // 2, core % 2
        wq, wk, wv, wo = wsets[h1]
        in_maps.append({"x": xf[b], "wq": wq, "wk": wk, "wv": wv, "wo": wo,
                        "ident": ident})

    res = run_bass_kernel_spmd(nc, in_maps, core_ids=list(range(N_CORES)))
    out = np.zeros((B, T, E), dtype=np.float32)
    for core in range(N_CORES):
        out[core // 2] += res.results[core]["out"]
    return out.reshape(B, P, Qd, E1, E2, E3)
